# revision 56
# baseline (speedup 1.0000x reference)
"""CapsuleNetwork (BiLSTM encoder + self-attention pooling + dynamic routing)
as a Trainium2 Bass/Tile kernel, SPMD data-parallel over 8 NeuronCores.

Sharding: batch B=128 split 16/core; weights replicated; no collectives.

Layout: feature dim on SBUF partitions, token index on the free dim.
x0/x1/zin use t-major columns (col = t*BS + b) so each recurrence step is a
contiguous 16-col slice; x2 stays b-major for the attention path.

LSTM step: a PE identity-matmul seeds the PSUM bank with zin (start=True),
the 16 W_hh matmuls accumulate on top (start=False), and one sigmoid over
all 8 gate blocks (g-gate rows pre-scaled x2 host-side; tanh(x)=2sig(2x)-1)
feeds a short elementwise chain.  The two directions are software-pipelined
half a step apart so each direction's matmul block overlaps the other's
elementwise chain.
"""

import sys

sys.path.insert(0, "/opt/trn_rl_repo")

import numpy as np
import ml_dtypes

BF16 = ml_dtypes.bfloat16

# problem dims
B, T, V, E, H, DA, R, SC, AT = 128, 64, 32000, 300, 256, 128, 8, 32, 16
NUM_ROUTING = 3
NCORES = 8
BS = B // NCORES          # 16 examples per core
TB = BS * T               # 1024 columns, t-major: col = t*BS + b
G4 = 4 * H                # 1024 gate rows
K0 = (128, 128, 44)       # layer-0 input chunk sizes (E=300)
KC1 = 4                   # layer-1 input chunks (2H=512)

# torch gate order i,f,g,o -> ours [i,f,o,g] (sigmoid block contiguous)
_PERM = np.concatenate([
    np.arange(0, 256), np.arange(256, 512), np.arange(768, 1024), np.arange(512, 768)
])


def _prep_wih0(w_ih):
    """[4H, 300] -> [128, 3, 1024] bf16 (transposed, gate-permuted, g x2)."""
    w = w_ih[_PERM].T.copy()              # [300, 1024]
    w[:, 768:] *= 2.0                     # sigmoid-trick scaling for g gates
    out = np.zeros((3 * 128, G4), np.float32)
    out[:E] = w
    return np.ascontiguousarray(
        out.reshape(3, 128, G4).transpose(1, 0, 2)).astype(BF16)


def _prep_wih1(w_ih):
    """[4H, 512] -> [128, 4, 1024] bf16."""
    w = w_ih[_PERM].T.copy()              # [512, 1024]
    w[:, 768:] *= 2.0
    return np.ascontiguousarray(
        w.reshape(KC1, 128, G4).transpose(1, 0, 2)).astype(BF16)


def _prep_whh(w_hh):
    """[4H, 256] -> [128, 2, 1024] bf16 (transposed, gate-permuted, g x2)."""
    w = w_hh[_PERM].T.copy()  # [256, 1024]
    w[:, 768:] *= 2.0
    return np.ascontiguousarray(
        w.reshape(2, 128, G4).transpose(1, 0, 2)).astype(BF16)


def _prep_bias(b):
    """[4H] -> [128, 8] f32 per-(m-block) per-partition bias, g x2."""
    bp = b[_PERM].astype(np.float32).copy()
    bp[768:] *= 2.0
    return np.ascontiguousarray(bp.reshape(8, 128).T)


def _host_prep(inputs):
    """Build the shared (replicated) arrays + per-core index arrays."""
    shared = {}

    emb = np.asarray(inputs["embedding"], np.float32)
    shared["emb"] = np.ascontiguousarray(emb).astype(BF16)

    for d, sfx in (("f", "f0"), ("b", "b0")):
        shared[f"wih0{d}"] = _prep_wih0(np.asarray(inputs[f"w_ih_{sfx}"], np.float32))
        shared[f"whh0{d}"] = _prep_whh(np.asarray(inputs[f"w_hh_{sfx}"], np.float32))
        shared[f"bias0{d}"] = _prep_bias(np.asarray(inputs[f"b_{sfx}"], np.float32))
    for d, sfx in (("f", "f1"), ("b", "b1")):
        shared[f"wih1{d}"] = _prep_wih1(np.asarray(inputs[f"w_ih_{sfx}"], np.float32))
        shared[f"whh1{d}"] = _prep_whh(np.asarray(inputs[f"w_hh_{sfx}"], np.float32))
        shared[f"bias1{d}"] = _prep_bias(np.asarray(inputs[f"b_{sfx}"], np.float32))

    ws1 = np.asarray(inputs["ws1"], np.float32)  # [128, 512]
    shared["ws1T"] = np.ascontiguousarray(
        ws1.T.reshape(4, 128, DA).transpose(1, 0, 2)).astype(BF16)
    shared["ws2T"] = np.ascontiguousarray(
        np.asarray(inputs["ws2"], np.float32).T).astype(BF16)  # [128, 8]

    cw = np.asarray(inputs["caps_w"], np.float32)  # [8, 512, 512]
    # -> [128, r=8, k=4, 512]
    shared["cw"] = np.ascontiguousarray(
        cw.reshape(R, 4, 128, SC * AT).transpose(2, 0, 1, 3)).astype(BF16)

    shared["ident"] = np.eye(128, dtype=np.float32).astype(BF16)

    ones_pre = np.zeros((128, BS), np.float32)   # [(r,b), b] block ones
    for r in range(R):
        for b in range(BS):
            ones_pre[r * BS + b, b] = 1.0
    shared["ones_pre"] = ones_pre.astype(BF16)
    shared["ones_rep"] = np.ascontiguousarray(ones_pre.T).astype(BF16)  # [b, (r,b)]

    tokens = np.asarray(inputs["tokens"]).astype(np.int64)  # [128, 64]
    idx_maps = []
    for c in range(NCORES):
        # t-major: token i = t*BS + b  ->  tokens[c*BS + b, t]
        flat = tokens[c * BS:(c + 1) * BS].T.reshape(-1)
        idx_maps.append(np.ascontiguousarray(
            flat.astype(np.int32).reshape(8, 128).T))
    return shared, idx_maps


# ---------------------------------------------------------------------------
# device program
# ---------------------------------------------------------------------------


def _declare_inputs(nc, mybir):
    dt = mybir.dt
    specs = {
        "emb": ((V, E), dt.bfloat16),
        "idx": ((128, 8), dt.int32),
        "wih0f": ((128, 3, G4), dt.bfloat16),
        "wih0b": ((128, 3, G4), dt.bfloat16),
        "whh0f": ((128, 2, G4), dt.bfloat16),
        "whh0b": ((128, 2, G4), dt.bfloat16),
        "bias0f": ((128, 8), dt.float32),
        "bias0b": ((128, 8), dt.float32),
        "wih1f": ((128, KC1, G4), dt.bfloat16),
        "wih1b": ((128, KC1, G4), dt.bfloat16),
        "whh1f": ((128, 2, G4), dt.bfloat16),
        "whh1b": ((128, 2, G4), dt.bfloat16),
        "bias1f": ((128, 8), dt.float32),
        "bias1b": ((128, 8), dt.float32),
        "ws1T": ((128, 4, DA), dt.bfloat16),
        "ws2T": ((128, R), dt.bfloat16),
        "cw": ((128, R, 4, SC * AT), dt.bfloat16),
        "ident": ((128, 128), dt.bfloat16),
        "ones_pre": ((128, BS), dt.bfloat16),
        "ones_rep": ((BS, 128), dt.bfloat16),
    }
    aps = {}
    for name, (shape, dtype) in specs.items():
        aps[name] = nc.dram_tensor(name, list(shape), dtype, kind="ExternalInput").ap()
    out = nc.dram_tensor("out", [BS, SC], mybir.dt.float32, kind="ExternalOutput").ap()
    return aps, out


def _zin_gemm(nc, mybir, ppool, wih_sb, x_sb, zin_sb, chunks, bias_sb):
    """zin[m, t, b] (bf16) = sum_k wihT[k][:,m-blk].T @ xT[k][:, cols] + bias.

    x_sb cols are t-major; zin is m-major so each PSUM half copies contiguous
    into zin[:, m, n*32:(n+1)*32, :].
    """
    f32 = mybir.dt.float32
    Alu = mybir.AluOpType
    AF = mybir.ActivationFunctionType
    nk = len(chunks)
    for m in range(8):
        # n-inner: consecutive matmul pairs share the same stationary
        # weights, halving the LDWEIGHTS pressure on the PE
        pss = [ppool.tile([128, 512], f32, tag=f"big{n}", bufs=3, name="zin_ps")
               for n in range(2)]
        for k, kk in enumerate(chunks):
            for n in range(2):
                nc.tensor.matmul(
                    pss[n][:],
                    wih_sb[0:kk, k, m * 128:(m + 1) * 128],
                    x_sb[0:kk, k, n * 512:(n + 1) * 512],
                    start=(k == 0), stop=(k == nk - 1))
        # bias-added evacuation, alternating engines so neither paces PE
        for n in range(2):
            if (m + n) % 2 == 0:
                nc.vector.tensor_scalar(
                    zin_sb[:, m, n * 32:(n + 1) * 32, :],
                    pss[n][:].rearrange("p (t b) -> p t b", b=BS),
                    bias_sb[:, m:m + 1], None, Alu.add)
            else:
                nc.scalar.activation(
                    zin_sb[:, m, n * 32:(n + 1) * 32, :],
                    pss[n][:].rearrange("p (t b) -> p t b", b=BS),
                    AF.Identity, bias=bias_sb[:, m:m + 1])


def _lstm_phase(nc, mybir, pools, whh_sb, zin_sb, xout_v, ident, dirs):
    """One BiLSTM layer, software-pipelined across the two directions.

    xout_v: [p, k, t, b] view of the output tile. dirs: [(dirname, koff)].
    """
    f32 = mybir.dt.float32
    AF = mybir.ActivationFunctionType
    Alu = mybir.AluOpType
    pspool, spool, state = pools["lstm_ps"], pools["step"], pools["state"]

    cst = {}
    for d, _ in dirs:
        c = state.tile([128, 32], f32, tag=f"c_{id(zin_sb[d])}_{d}",
                       name=f"c_{d}")
        nc.vector.memset(c[:], 0.0)
        cst[d] = c

    ps_t = {}
    ps_q = {d: [] for d, _ in dirs}

    def emit_icopy(d, step):
        # PSUM seed for `step`, emitted one step ahead so it lands in the PE
        # FIFO inside this direction's chain-wait window instead of right
        # before the W-block that depends on h.
        tt = step if d == "f" else T - 1 - step
        ps = pspool.tile([128, 512], f32, tag=f"ps_{d}", name=f"ps_{d}")
        nc.tensor.matmul(ps[:, 0:128], ident[:], zin_sb[d][:, :, tt, :],
                         start=True, stop=(step == 0))
        ps_q[d].append(ps)

    def emit_w_block(d, koff, step):
        tt = step if d == "f" else T - 1 - step
        tprev = tt - 1 if d == "f" else tt + 1
        ps = ps_q[d].pop(0)
        ps_t[d] = ps
        if step > 0:
            for m in range(8):
                for k in range(2):
                    nc.tensor.matmul(
                        ps[:, m * 16:(m + 1) * 16],
                        whh_sb[d][:, k, m * 128:(m + 1) * 128],
                        xout_v[:, koff + k, tprev, :],
                        start=False, stop=(m == 7 and k == 1))

    def emit_ew(d, koff, step):
        tt = step if d == "f" else T - 1 - step
        ps, c = ps_t[d], cst[d]
        s = spool.tile([128, 8, BS], f32, tag=f"s_{d}", name=f"s_{d}")
        nc.scalar.activation(s[:], ps[:, 0:128].rearrange("p (m b) -> p m b", b=BS),
                             AF.Sigmoid)
        t1 = spool.tile([128, 32], f32, tag=f"t1_{d}", name=f"t1_{d}")
        nc.gpsimd.tensor_tensor(
            t1[:], s[:, 2:4, :].rearrange("p m b -> p (m b)"), c[:], Alu.mult)
        # w = (sig_g - 0.5) * sig_i; c = 2w + sig_f*c  ==  sig_i*tanh(g) + sig_f*c
        w = spool.tile([128, 32], f32, tag=f"w_{d}", name=f"w_{d}")
        nc.vector.scalar_tensor_tensor(
            w[:], s[:, 6:8, :].rearrange("p m b -> p (m b)"), 0.5,
            s[:, 0:2, :].rearrange("p m b -> p (m b)"), Alu.subtract, Alu.mult)
        nc.vector.scalar_tensor_tensor(c[:], w[:], 2.0, t1[:], Alu.mult, Alu.add)
        # th lives in PSUM (own bank): ScalarE writes PSUM faster than SBUF,
        # and tanh -> h is on the recurrence critical chain
        th = pools["th"].tile([128, 512], f32, tag=f"th_{d}", name=f"th_{d}")
        nc.scalar.activation(th[:, 0:32], c[:], AF.Tanh)
        nc.vector.tensor_tensor(
            xout_v[:, koff:koff + 2, tt, :], s[:, 4:6, :],
            th[:, 0:32].rearrange("p (m b) -> p m b", b=BS), Alu.mult)

    (df, kf), (db, kb) = dirs
    emit_icopy(df, 0)
    emit_icopy(db, 0)
    for step in range(T):
        emit_w_block(df, kf, step)
        if step > 0:
            emit_ew(db, kb, step - 1)
        emit_w_block(db, kb, step)
        # both seeds back-to-back on the PE so the second shares the
        # identity LDWEIGHTS with the first
        if step + 1 < T:
            emit_icopy(df, step + 1)
            emit_icopy(db, step + 1)
        emit_ew(df, kf, step)
    emit_ew(db, kb, T - 1)


def _body(nc, tc, mybir, ins, out):
    import contextlib
    dt = mybir.dt
    AF = mybir.ActivationFunctionType
    Alu = mybir.AluOpType
    bf16, f32 = dt.bfloat16, dt.float32

    ctx = contextlib.ExitStack()
    with ctx:
        persist = ctx.enter_context(tc.tile_pool(name="persist", bufs=1))
        wpool = ctx.enter_context(tc.tile_pool(name="weights", bufs=1))
        zpool = ctx.enter_context(tc.tile_pool(name="zin", bufs=1))
        state = ctx.enter_context(tc.tile_pool(name="state", bufs=1))
        step = ctx.enter_context(tc.tile_pool(name="step", bufs=3))
        pools = {"state": state, "step": step}

        # ---- load indices + gather embedding rows, transpose on PE ----
        import concourse.bass as bass
        idx_sb = persist.tile([128, 8], dt.int32)
        nc.sync.dma_start(idx_sb[:], ins["idx"])
        ident = persist.tile([128, 128], bf16)
        nc.sync.dma_start(ident[:], ins["ident"])
        # prefetch the big capsule weights early so they overlap the LSTMs
        cw = persist.tile([128, R, 4, SC * AT], bf16, tag="cw", name="cw")
        nc.scalar.dma_start(cw[:], ins["cw"])
        ws1T = persist.tile([128, 4, DA], bf16)
        ws2T = persist.tile([128, R], bf16)
        nc.sync.dma_start(ws1T[:], ins["ws1T"])
        nc.sync.dma_start(ws2T[:], ins["ws2T"])

        # tiny dummy sigmoid first so the initial ACT table load picks the
        # sigmoid set (avoids a ~2.7us set switch at the LSTM0 boundary)
        warm = persist.tile([1, 1], f32, tag="warm", name="warm")
        nc.vector.memset(warm[:], 0.0)
        nc.scalar.activation(warm[:], warm[:], AF.Sigmoid)

        x0rows = persist.tile([128, 8, E], bf16, tag="x0rows", name="x0rows")
        for j in range(8):
            nc.gpsimd.indirect_dma_start(
                out=x0rows[:, j, :], out_offset=None, in_=ins["emb"],
                in_offset=bass.IndirectOffsetOnAxis(ap=idx_sb[:, j:j + 1], axis=0))

        with tc.tile_pool(name="ps_g0", bufs=6, space="PSUM") as ppool:
            x0 = persist.tile([128, 3, TB], bf16, tag="x0", name="x0")
            # c-outer: bunches the transposes ahead of the GEMM so the GEMM
            # runs as one dense matmul burst (PE transposes don't count as
            # busy for the HAM clock gate; interleaving them keeps PE cold)
            for c in range(3):
                kk = K0[c]
                for j in range(8):
                    pst = ppool.tile([128, 128], bf16, tag="tr", bufs=2, name="g_tr")
                    nc.tensor.transpose(pst[0:kk, :],
                                        x0rows[:, j, c * 128:c * 128 + kk], ident[:])
                    nc.vector.tensor_copy(out=x0[0:kk, c, j * 128:(j + 1) * 128],
                                          in_=pst[0:kk, :])

            # ---- layer-0 weights + input GEMMs ----
            wih0 = {d: wpool.tile([128, 3, G4], bf16, tag=f"wih0{d}", name=f"wih0{d}") for d in "fb"}
            whh0 = {d: wpool.tile([128, 2, G4], bf16, tag=f"whh0{d}", name=f"whh0{d}") for d in "fb"}
            bias0 = {d: wpool.tile([128, 8], f32, tag=f"bias0{d}", name=f"bias0{d}") for d in "fb"}
            for d in "fb":
                nc.sync.dma_start(wih0[d][:], ins[f"wih0{d}"])
                nc.sync.dma_start(whh0[d][:], ins[f"whh0{d}"])
                nc.sync.dma_start(bias0[d][:], ins[f"bias0{d}"])
            zin0 = {d: zpool.tile([128, 8, T, BS], bf16, tag=f"zin{d}", name=f"zin0{d}") for d in "fb"}
            for d in "fb":
                _zin_gemm(nc, mybir, ppool, wih0[d], x0, zin0[d], K0, bias0[d])

        # ---- layer-0 recurrence -> x1 (t-major; chunks f:0-1 b:2-3) ----
        x1 = persist.tile([128, KC1, TB], bf16, tag="x1", name="x1")
        xv1 = x1[:].rearrange("p k (t b) -> p k t b", b=BS)
        with tc.tile_pool(name="ps_l0", bufs=3, space="PSUM") as lstm_ps, \
                tc.tile_pool(name="ps_th0", bufs=1, space="PSUM") as thpool:
            pools["lstm_ps"] = lstm_ps
            pools["th"] = thpool
            _lstm_phase(nc, mybir, pools, whh0, zin0, xv1, ident,
                        [("f", 0), ("b", 2)])

        # ---- layer-1 input GEMMs + recurrence -> x2 (b-major) ----
        wih1 = {d: wpool.tile([128, KC1, G4], bf16, tag=f"wih1{d}", name=f"wih1{d}") for d in "fb"}
        whh1 = {d: wpool.tile([128, 2, G4], bf16, tag=f"whh1{d}", name=f"whh1{d}") for d in "fb"}
        bias1 = {d: wpool.tile([128, 8], f32, tag=f"bias1{d}", name=f"bias1{d}") for d in "fb"}
        for d in "fb":
            nc.sync.dma_start(wih1[d][:], ins[f"wih1{d}"])
            nc.sync.dma_start(whh1[d][:], ins[f"whh1{d}"])
            nc.sync.dma_start(bias1[d][:], ins[f"bias1{d}"])
        zin1 = {d: zpool.tile([128, 8, T, BS], bf16, tag=f"zin{d}", name=f"zin1{d}") for d in "fb"}
        with tc.tile_pool(name="ps_g1", bufs=8, space="PSUM") as ppool:
            for d in "fb":
                _zin_gemm(nc, mybir, ppool, wih1[d], x1, zin1[d], (128,) * KC1, bias1[d])
        x2 = persist.tile([128, 4, TB], bf16, tag="x2", name="x2")
        xv2 = x2[:].rearrange("p k (b t) -> p k t b", t=T)
        with tc.tile_pool(name="ps_l1", bufs=3, space="PSUM") as lstm_ps, \
                tc.tile_pool(name="ps_th1", bufs=1, space="PSUM") as thpool:
            pools["lstm_ps"] = lstm_ps
            pools["th"] = thpool
            _lstm_phase(nc, mybir, pools, whh1, zin1, xv2, ident,
                        [("f", 0), ("b", 2)])

        psum_big = ctx.enter_context(tc.tile_pool(name="psum_tail", bufs=6, space="PSUM"))

        # ---- x2row[(b t), u] via PE transposes ----
        x2row = persist.tile([128, 8, 512], bf16, tag="x2row", name="x2row")
        for c in range(4):
            for j in range(8):
                pst = psum_big.tile([128, 128], bf16, tag="big", name="tr_ps")
                nc.tensor.transpose(pst[:], x2[:, c, j * 128:(j + 1) * 128], ident[:])
                if j % 2 == 0:
                    nc.vector.tensor_copy(out=x2row[:, j, c * 128:(c + 1) * 128],
                                          in_=pst[:])
                else:
                    nc.scalar.copy(x2row[:, j, c * 128:(c + 1) * 128], pst[:])

        # ---- attention: hbar = tanh(ws1 @ x2T) [DA, TB] ----
        hbar = persist.tile([128, TB], bf16, tag="hbar", name="hbar")
        hps = [psum_big.tile([128, 512], f32, tag=f"hb{n}", bufs=1, name="hb_ps")
               for n in range(2)]
        for k in range(4):
            for n in range(2):
                nc.tensor.matmul(hps[n][:], ws1T[:, k, :],
                                 x2[:, k, n * 512:(n + 1) * 512],
                                 start=(k == 0), stop=(k == 3))
        for n in range(2):
            nc.scalar.activation(hbar[:, n * 512:(n + 1) * 512], hps[n][:], AF.Tanh)

        # ---- att[b,r,t] then block-diagonal att2 [(b t), (b r)] ----
        att_ps = psum_big.tile([128, 8, R], f32, tag="big", name="att_ps")
        for bp in range(8):
            nc.tensor.matmul(att_ps[:, bp, :], hbar[:, bp * 128:(bp + 1) * 128],
                             ws2T[:], start=True, stop=True)
        att2 = persist.tile([128, 8, 128], bf16, tag="att2", name="att2")
        nc.vector.memset(att2[:], 0.0)
        for bp in range(8):
            nc.vector.tensor_copy(out=att2[0:64, bp, bp * 16:bp * 16 + 8],
                                  in_=att_ps[0:64, bp, :])
            nc.vector.tensor_copy(out=att2[64:128, bp, bp * 16 + 8:bp * 16 + 16],
                                  in_=att_ps[64:128, bp, :])

        # ---- sentT [u, (b r)] = x2row.T @ att2 ----
        sentT = persist.tile([128, 4, 128], bf16, tag="sentT", name="sentT")
        for c in range(4):
            ps = psum_big.tile([128, 128], f32, tag="big", name="sent_ps")
            for po in range(8):
                nc.tensor.matmul(ps[:], x2row[:, po, c * 128:(c + 1) * 128],
                                 att2[:, po, :], start=(po == 0), stop=(po == 7))
            nc.vector.tensor_copy(out=sentT[:, c, :], in_=ps[:])

        # ---- votes [(r b), (c a)] (bf16 for the routing matmuls) ----
        votes = persist.tile([128, SC * AT], bf16, tag="votes", name="votes")
        vstage = zpool.tile([BS, R, SC * AT], bf16, tag="zinf", name="vstage")
        sentv = sentT[:].rearrange("p k (b r) -> p k r b", r=R)
        dma_eng = [nc.sync, nc.scalar]
        for r in range(R):
            ps = psum_big.tile([BS, 512], f32, tag="big", name="vote_ps")
            for k in range(4):
                nc.tensor.matmul(ps[:], sentv[:, k, r, :], cw[:, r, k, :],
                                 start=(k == 0), stop=(k == 3))
            nc.vector.tensor_copy(out=vstage[:, r, :], in_=ps[:])
        for r in range(R):
            dma_eng[r % 2].dma_start(votes[r * BS:(r + 1) * BS, :], vstage[:, r, :])

        # ---- dynamic routing ----
        ones_pre = persist.tile([128, BS], bf16)
        ones_rep = persist.tile([BS, 128], bf16)
        nc.sync.dma_start(ones_pre[:], ins["ones_pre"])
        nc.sync.dma_start(ones_rep[:], ins["ones_rep"])
        votes_v = votes[:].rearrange("p (c a) -> p c a", a=AT)

        i32 = dt.int32
        MAGIC = 0x5F3759DF

        def rsqrt(pool, x, n):
            """Quake rsqrt on DVE (avoids Sqrt table-set thrash on the
            scalar engine). No Newton step: ~3.4% error only perturbs the
            routing attention weights; the output norm path is exact."""
            t = pool.tile([BS, n], i32, tag="rs_t", name="rs_t")
            nc.vector.tensor_scalar(t[:], x.bitcast(i32), 1, None,
                                    Alu.logical_shift_right)
            j = pool.tile([BS, n], i32, tag="rs_j", name="rs_j")
            nc.vector.tensor_scalar(j[:], t[:], -1, MAGIC, Alu.mult, Alu.add)
            return j[:].bitcast(f32)

        rpool = ctx.enter_context(tc.tile_pool(name="routing", bufs=2))
        logits = None
        n2 = dinv = None
        for it in range(NUM_ROUTING):
            if it == 0:
                route = rpool.tile([128, SC], f32, tag="route", name="route")
                nc.vector.memset(route[:], 1.0 / SC)
            else:
                # logits are O(1): skip the max-subtraction, exp is safe
                e = rpool.tile([128, SC], f32, tag="e", name="e")
                ssum = rpool.tile([128, 1], f32, tag="ssum", name="ssum")
                nc.scalar.activation(e[:], logits[:], AF.Exp, accum_out=ssum[:])
                sinv = rpool.tile([128, 1], f32, tag="sinv", name="sinv")
                nc.vector.reciprocal(sinv[:], ssum[:])
                route = rpool.tile([128, SC], f32, tag="route", name="route")
                nc.vector.tensor_scalar_mul(route[:], e[:], sinv[:])
            tmp = rpool.tile([128, SC, AT], bf16, tag="tmp", name="tmp")
            nc.vector.tensor_tensor(
                tmp[:], votes_v,
                route[:, :, None].to_broadcast((128, SC, AT)), Alu.mult)
            pre = psum_big.tile([BS, SC * AT], f32, tag="big", name="pre_ps")
            nc.tensor.matmul(pre[:], ones_pre[:],
                             tmp[:].rearrange("p c a -> p (c a)"),
                             start=True, stop=True)
            sq = rpool.tile([BS, SC, AT], f32, tag="sq", name="sq")
            nc.scalar.activation(sq[:], pre[:].rearrange("p (c a) -> p c a", a=AT),
                                 AF.Square)
            n2 = rpool.tile([BS, SC], f32, tag="n2", name="n2")
            nc.vector.tensor_reduce(n2[:], sq[:], mybir.AxisListType.X, Alu.add)
            den = rpool.tile([BS, SC], f32, tag="den", name="den")
            nc.vector.tensor_scalar_add(den[:], n2[:], 0.5)
            dinv = rpool.tile([BS, SC], f32, tag="dinv", name="dinv")
            nc.vector.reciprocal(dinv[:], den[:])
            if it < NUM_ROUTING - 1:
                ry = rsqrt(rpool, n2[:], SC)
                s1 = rpool.tile([BS, SC], f32, tag="s1", name="s1")
                nc.vector.tensor_tensor(s1[:], n2[:], dinv[:], Alu.mult)
                nc.vector.tensor_tensor(s1[:], s1[:], ry, Alu.mult)
                act = rpool.tile([BS, SC, AT], bf16, tag="act", name="act")
                nc.vector.tensor_tensor(
                    act[:], pre[:].rearrange("p (c a) -> p c a", a=AT),
                    s1[:, :, None].to_broadcast((BS, SC, AT)), Alu.mult)
                rep = psum_big.tile([128, SC * AT], f32, tag="big", name="rep_ps")
                nc.tensor.matmul(rep[:], ones_rep[:],
                                 act[:].rearrange("p c a -> p (c a)"),
                                 start=True, stop=True)
                u = rpool.tile([128, SC, AT], f32, tag="u", name="u")
                nc.vector.tensor_tensor(
                    u[:], votes_v, rep[:].rearrange("p (c a) -> p c a", a=AT),
                    Alu.mult)
                dl = rpool.tile([128, SC], f32, tag="dl", name="dl")
                nc.vector.tensor_reduce(dl[:], u[:], mybir.AxisListType.X, Alu.add)
                if it == 0:
                    logits = dl
                else:
                    new_logits = rpool.tile([128, SC], f32, tag="logits", name="logits")
                    nc.vector.tensor_add(new_logits[:], logits[:], dl[:])
                    logits = new_logits

        outsb = persist.tile([BS, SC], f32, tag="outsb", name="outsb")
        nc.vector.tensor_tensor(outsb[:], n2[:], dinv[:], Alu.mult)
        nc.sync.dma_start(out, outsb[:])


_CACHED = {}


def _build():
    if "nc" in _CACHED:
        return _CACHED["nc"], _CACHED["ins"]
    import concourse.bacc as bacc
    import concourse.tile as tile
    import concourse.mybir as mybir
    from concourse._compat import axon_active  # noqa: F401

    nc = bacc.Bacc("TRN2", target_bir_lowering=False, debug=False)
    ins, out = _declare_inputs(nc, mybir)
    with tile.TileContext(nc) as tc:
        _body(nc, tc, mybir, ins, out)
    nc.compile()
    _CACHED["nc"] = nc
    _CACHED["ins"] = ins
    return nc, ins


def kernel(**inputs):
    from concourse.bass_utils import run_bass_kernel_spmd

    shared, idx_maps = _host_prep(inputs)
    nc, _ = _build()
    in_maps = []
    for c in range(NCORES):
        m = dict(shared)
        m["idx"] = idx_maps[c]
        in_maps.append(m)
    res = run_bass_kernel_spmd(nc, in_maps, core_ids=list(range(NCORES)))
    out = np.concatenate([res.results[c]["out"] for c in range(NCORES)], axis=0)
    return out.astype(np.float32)


# revision 59
# speedup vs baseline: 1.1899x; 1.1899x over previous
"""CapsuleNetwork (BiLSTM encoder + self-attention pooling + dynamic routing)
as a Trainium2 Bass/Tile kernel, SPMD data-parallel over 8 NeuronCores.

Sharding: batch B=128 split 16/core; weights replicated; no collectives.

Layout: feature dim on SBUF partitions, token index on the free dim.
x0/x1/zin use t-major columns (col = t*BS + b) so each recurrence step is a
contiguous 16-col slice; x2 stays b-major for the attention path.

LSTM step: a PE identity-matmul seeds the PSUM bank with zin (start=True),
the 16 W_hh matmuls accumulate on top (start=False), and one sigmoid over
all 8 gate blocks (g-gate rows pre-scaled x2 host-side; tanh(x)=2sig(2x)-1)
feeds a short elementwise chain.  The two directions are software-pipelined
half a step apart so each direction's matmul block overlaps the other's
elementwise chain.
"""

import sys

sys.path.insert(0, "/opt/trn_rl_repo")

import numpy as np
import ml_dtypes

BF16 = ml_dtypes.bfloat16

# problem dims
B, T, V, E, H, DA, R, SC, AT = 128, 64, 32000, 300, 256, 128, 8, 32, 16
NUM_ROUTING = 3
NCORES = 8
BS = B // NCORES          # 16 examples per core
TB = BS * T               # 1024 columns, t-major: col = t*BS + b
G4 = 4 * H                # 1024 gate rows
K0 = (128, 128, 44)       # layer-0 input chunk sizes (E=300)
KC1 = 4                   # layer-1 input chunks (2H=512)

# torch gate order i,f,g,o -> ours [i,f,o,g] (sigmoid block contiguous)
_PERM = np.concatenate([
    np.arange(0, 256), np.arange(256, 512), np.arange(768, 1024), np.arange(512, 768)
])


def _prep_wih0(w_ih):
    """[4H, 300] -> [128, 3, 1024] bf16 (transposed, gate-permuted, g x2)."""
    w = w_ih[_PERM].T.copy()              # [300, 1024]
    w[:, 768:] *= 2.0                     # sigmoid-trick scaling for g gates
    out = np.zeros((3 * 128, G4), np.float32)
    out[:E] = w
    return np.ascontiguousarray(
        out.reshape(3, 128, G4).transpose(1, 0, 2)).astype(BF16)


def _prep_wih1(w_ih):
    """[4H, 512] -> [128, 4, 1024] bf16."""
    w = w_ih[_PERM].T.copy()              # [512, 1024]
    w[:, 768:] *= 2.0
    return np.ascontiguousarray(
        w.reshape(KC1, 128, G4).transpose(1, 0, 2)).astype(BF16)


def _prep_whh(w_hh):
    """[4H, 256] -> [128, 2, 1024] bf16 (transposed, gate-permuted, g x2)."""
    w = w_hh[_PERM].T.copy()  # [256, 1024]
    w[:, 768:] *= 2.0
    return np.ascontiguousarray(
        w.reshape(2, 128, G4).transpose(1, 0, 2)).astype(BF16)


def _prep_bias(b):
    """[4H] -> [128, 8] f32 per-(m-block) per-partition bias, g x2."""
    bp = b[_PERM].astype(np.float32).copy()
    bp[768:] *= 2.0
    return np.ascontiguousarray(bp.reshape(8, 128).T)


def _host_prep(inputs):
    """Build the shared (replicated) arrays + per-core index arrays."""
    shared = {}

    emb = np.asarray(inputs["embedding"], np.float32)
    shared["emb"] = np.ascontiguousarray(emb).astype(BF16)

    for d, sfx in (("f", "f0"), ("b", "b0")):
        shared[f"wih0{d}"] = _prep_wih0(np.asarray(inputs[f"w_ih_{sfx}"], np.float32))
        shared[f"whh0{d}"] = _prep_whh(np.asarray(inputs[f"w_hh_{sfx}"], np.float32))
        shared[f"bias0{d}"] = _prep_bias(np.asarray(inputs[f"b_{sfx}"], np.float32))
    for d, sfx in (("f", "f1"), ("b", "b1")):
        shared[f"wih1{d}"] = _prep_wih1(np.asarray(inputs[f"w_ih_{sfx}"], np.float32))
        shared[f"whh1{d}"] = _prep_whh(np.asarray(inputs[f"w_hh_{sfx}"], np.float32))
        shared[f"bias1{d}"] = _prep_bias(np.asarray(inputs[f"b_{sfx}"], np.float32))

    ws1 = np.asarray(inputs["ws1"], np.float32)  # [128, 512]
    shared["ws1T"] = np.ascontiguousarray(
        ws1.T.reshape(4, 128, DA).transpose(1, 0, 2)).astype(BF16)
    shared["ws2T"] = np.ascontiguousarray(
        np.asarray(inputs["ws2"], np.float32).T).astype(BF16)  # [128, 8]

    cw = np.asarray(inputs["caps_w"], np.float32)  # [8, 512, 512]
    # -> [128, r=8, k=4, 512]
    shared["cw"] = np.ascontiguousarray(
        cw.reshape(R, 4, 128, SC * AT).transpose(2, 0, 1, 3)).astype(BF16)

    shared["ident"] = np.eye(128, dtype=np.float32).astype(BF16)

    ones_pre = np.zeros((128, BS), np.float32)   # [(r,b), b] block ones
    for r in range(R):
        for b in range(BS):
            ones_pre[r * BS + b, b] = 1.0
    shared["ones_pre"] = ones_pre.astype(BF16)
    shared["ones_rep"] = np.ascontiguousarray(ones_pre.T).astype(BF16)  # [b, (r,b)]

    tokens = np.asarray(inputs["tokens"]).astype(np.int64)  # [128, 64]
    idx_maps = []
    for c in range(NCORES):
        # t-major: token i = t*BS + b  ->  tokens[c*BS + b, t]
        flat = tokens[c * BS:(c + 1) * BS].T.reshape(-1)
        idx_maps.append(np.ascontiguousarray(
            flat.astype(np.int32).reshape(8, 128).T))
    return shared, idx_maps


# ---------------------------------------------------------------------------
# device program
# ---------------------------------------------------------------------------


def _declare_inputs(nc, mybir):
    dt = mybir.dt
    specs = {
        "emb": ((V, E), dt.bfloat16),
        "idx": ((128, 8), dt.int32),
        "wih0f": ((128, 3, G4), dt.bfloat16),
        "wih0b": ((128, 3, G4), dt.bfloat16),
        "whh0f": ((128, 2, G4), dt.bfloat16),
        "whh0b": ((128, 2, G4), dt.bfloat16),
        "bias0f": ((128, 8), dt.float32),
        "bias0b": ((128, 8), dt.float32),
        "wih1f": ((128, KC1, G4), dt.bfloat16),
        "wih1b": ((128, KC1, G4), dt.bfloat16),
        "whh1f": ((128, 2, G4), dt.bfloat16),
        "whh1b": ((128, 2, G4), dt.bfloat16),
        "bias1f": ((128, 8), dt.float32),
        "bias1b": ((128, 8), dt.float32),
        "ws1T": ((128, 4, DA), dt.bfloat16),
        "ws2T": ((128, R), dt.bfloat16),
        "cw": ((128, R, 4, SC * AT), dt.bfloat16),
        "ident": ((128, 128), dt.bfloat16),
        "ones_pre": ((128, BS), dt.bfloat16),
        "ones_rep": ((BS, 128), dt.bfloat16),
    }
    aps = {}
    for name, (shape, dtype) in specs.items():
        aps[name] = nc.dram_tensor(name, list(shape), dtype, kind="ExternalInput").ap()
    out = nc.dram_tensor("out", [BS, SC], mybir.dt.float32, kind="ExternalOutput").ap()
    return aps, out


def _zin_gemm(nc, mybir, ppool, wih_sb, x_sb, zin_sb, chunks, bias_sb):
    """zin[m, t, b] (bf16) = sum_k wihT[k][:,m-blk].T @ xT[k][:, cols] + bias.

    x_sb cols are t-major; zin is m-major so each PSUM half copies contiguous
    into zin[:, m, n*32:(n+1)*32, :].
    """
    f32 = mybir.dt.float32
    Alu = mybir.AluOpType
    AF = mybir.ActivationFunctionType
    nk = len(chunks)
    for m in range(8):
        # n-inner: consecutive matmul pairs share the same stationary
        # weights, halving the LDWEIGHTS pressure on the PE
        pss = [ppool.tile([128, 512], f32, tag=f"big{n}", bufs=3, name="zin_ps")
               for n in range(2)]
        for k, kk in enumerate(chunks):
            for n in range(2):
                nc.tensor.matmul(
                    pss[n][:],
                    wih_sb[0:kk, k, m * 128:(m + 1) * 128],
                    x_sb[0:kk, k, n * 512:(n + 1) * 512],
                    start=(k == 0), stop=(k == nk - 1))
        # bias-added evacuation, alternating engines so neither paces PE
        for n in range(2):
            if (m + n) % 2 == 0:
                nc.vector.tensor_scalar(
                    zin_sb[:, m, n * 32:(n + 1) * 32, :],
                    pss[n][:].rearrange("p (t b) -> p t b", b=BS),
                    bias_sb[:, m:m + 1], None, Alu.add)
            else:
                nc.scalar.activation(
                    zin_sb[:, m, n * 32:(n + 1) * 32, :],
                    pss[n][:].rearrange("p (t b) -> p t b", b=BS),
                    AF.Identity, bias=bias_sb[:, m:m + 1])


def _lstm_phase(nc, mybir, pools, whh_sb, zin_sb, xout_v, ident, dirs):
    """One BiLSTM layer, software-pipelined across the two directions.

    xout_v: [p, k, t, b] view of the output tile. dirs: [(dirname, koff)].
    """
    f32 = mybir.dt.float32
    AF = mybir.ActivationFunctionType
    Alu = mybir.AluOpType
    pspool, spool, state = pools["lstm_ps"], pools["step"], pools["state"]

    cst = {}
    for d, _ in dirs:
        c = state.tile([128, 32], f32, tag=f"c_{id(zin_sb[d])}_{d}",
                       name=f"c_{d}")
        nc.vector.memset(c[:], 0.0)
        cst[d] = c

    ps_t = {}
    ps_q = {d: [] for d, _ in dirs}

    def emit_icopy(d, step):
        # PSUM seed for `step`, emitted one step ahead so it lands in the PE
        # FIFO inside this direction's chain-wait window instead of right
        # before the W-block that depends on h.
        tt = step if d == "f" else T - 1 - step
        ps = pspool.tile([128, 512], f32, tag=f"ps_{d}", name=f"ps_{d}")
        nc.tensor.matmul(ps[:, 0:128], ident[:], zin_sb[d][:, :, tt, :],
                         start=True, stop=(step == 0))
        ps_q[d].append(ps)

    def emit_w_block(d, koff, step):
        tt = step if d == "f" else T - 1 - step
        tprev = tt - 1 if d == "f" else tt + 1
        ps = ps_q[d].pop(0)
        ps_t[d] = ps
        if step > 0:
            for m in range(8):
                for k in range(2):
                    nc.tensor.matmul(
                        ps[:, m * 16:(m + 1) * 16],
                        whh_sb[d][:, k, m * 128:(m + 1) * 128],
                        xout_v[:, koff + k, tprev, :],
                        start=False, stop=(m == 7 and k == 1))

    def emit_ew(d, koff, step):
        tt = step if d == "f" else T - 1 - step
        ps, c = ps_t[d], cst[d]
        s = spool.tile([128, 8, BS], f32, tag=f"s_{d}", name=f"s_{d}")
        nc.scalar.activation(s[:], ps[:, 0:128].rearrange("p (m b) -> p m b", b=BS),
                             AF.Sigmoid)
        t1 = spool.tile([128, 32], f32, tag=f"t1_{d}", name=f"t1_{d}")
        nc.gpsimd.tensor_tensor(
            t1[:], s[:, 2:4, :].rearrange("p m b -> p (m b)"), c[:], Alu.mult)
        # w = (sig_g - 0.5) * sig_i; c = 2w + sig_f*c  ==  sig_i*tanh(g) + sig_f*c
        w = spool.tile([128, 32], f32, tag=f"w_{d}", name=f"w_{d}")
        nc.vector.scalar_tensor_tensor(
            w[:], s[:, 6:8, :].rearrange("p m b -> p (m b)"), 0.5,
            s[:, 0:2, :].rearrange("p m b -> p (m b)"), Alu.subtract, Alu.mult)
        nc.vector.scalar_tensor_tensor(c[:], w[:], 2.0, t1[:], Alu.mult, Alu.add)
        # th lives in PSUM (own bank): ScalarE writes PSUM faster than SBUF,
        # and tanh -> h is on the recurrence critical chain
        th = pools["th"].tile([128, 512], f32, tag=f"th_{d}", name=f"th_{d}")
        nc.scalar.activation(th[:, 0:32], c[:], AF.Tanh)
        nc.vector.tensor_tensor(
            xout_v[:, koff:koff + 2, tt, :], s[:, 4:6, :],
            th[:, 0:32].rearrange("p (m b) -> p m b", b=BS), Alu.mult)

    (df, kf), (db, kb) = dirs
    emit_icopy(df, 0)
    emit_icopy(db, 0)
    for step in range(T):
        emit_w_block(df, kf, step)
        if step > 0:
            emit_ew(db, kb, step - 1)
        emit_w_block(db, kb, step)
        # both seeds back-to-back on the PE so the second shares the
        # identity LDWEIGHTS with the first
        if step + 1 < T:
            emit_icopy(df, step + 1)
            emit_icopy(db, step + 1)
        emit_ew(df, kf, step)
    emit_ew(db, kb, T - 1)


def _body(nc, tc, mybir, ins, out):
    import contextlib
    dt = mybir.dt
    AF = mybir.ActivationFunctionType
    Alu = mybir.AluOpType
    bf16, f32 = dt.bfloat16, dt.float32

    ctx = contextlib.ExitStack()
    with ctx:
        persist = ctx.enter_context(tc.tile_pool(name="persist", bufs=1))
        wpool = ctx.enter_context(tc.tile_pool(name="weights", bufs=1))
        zpool = ctx.enter_context(tc.tile_pool(name="zin", bufs=1))
        state = ctx.enter_context(tc.tile_pool(name="state", bufs=1))
        step = ctx.enter_context(tc.tile_pool(name="step", bufs=3))
        pools = {"state": state, "step": step}

        # ---- load indices + gather embedding rows, transpose on PE ----
        import concourse.bass as bass
        idx_sb = persist.tile([128, 8], dt.int32)
        nc.sync.dma_start(idx_sb[:], ins["idx"])
        ident = persist.tile([128, 128], bf16)
        nc.sync.dma_start(ident[:], ins["ident"])
        # prefetch the big capsule weights early so they overlap the LSTMs
        cw = persist.tile([128, R, 4, SC * AT], bf16, tag="cw", name="cw")
        nc.scalar.dma_start(cw[:], ins["cw"])
        ws1T = persist.tile([128, 4, DA], bf16)
        ws2T = persist.tile([128, R], bf16)
        nc.sync.dma_start(ws1T[:], ins["ws1T"])
        nc.sync.dma_start(ws2T[:], ins["ws2T"])

        # tiny dummy sigmoid first so the initial ACT table load picks the
        # sigmoid set (avoids a ~2.7us set switch at the LSTM0 boundary)
        warm = persist.tile([1, 1], f32, tag="warm", name="warm")
        nc.vector.memset(warm[:], 0.0)
        nc.scalar.activation(warm[:], warm[:], AF.Sigmoid)

        x0rows = persist.tile([128, 8, E], bf16, tag="x0rows", name="x0rows")
        for j in range(8):
            nc.gpsimd.indirect_dma_start(
                out=x0rows[:, j, :], out_offset=None, in_=ins["emb"],
                in_offset=bass.IndirectOffsetOnAxis(ap=idx_sb[:, j:j + 1], axis=0))

        with tc.tile_pool(name="ps_g0", bufs=6, space="PSUM") as ppool:
            x0 = persist.tile([128, 3, TB], bf16, tag="x0", name="x0")
            # c-outer: bunches the transposes ahead of the GEMM so the GEMM
            # runs as one dense matmul burst (PE transposes don't count as
            # busy for the HAM clock gate; interleaving them keeps PE cold)
            for c in range(3):
                kk = K0[c]
                for j in range(8):
                    pst = ppool.tile([128, 128], bf16, tag="tr", bufs=2, name="g_tr")
                    nc.tensor.transpose(pst[0:kk, :],
                                        x0rows[:, j, c * 128:c * 128 + kk], ident[:])
                    nc.vector.tensor_copy(out=x0[0:kk, c, j * 128:(j + 1) * 128],
                                          in_=pst[0:kk, :])

            # ---- layer-0 weights + input GEMMs ----
            wih0 = {d: wpool.tile([128, 3, G4], bf16, tag=f"wih0{d}", name=f"wih0{d}") for d in "fb"}
            whh0 = {d: wpool.tile([128, 2, G4], bf16, tag=f"whh0{d}", name=f"whh0{d}") for d in "fb"}
            bias0 = {d: wpool.tile([128, 8], f32, tag=f"bias0{d}", name=f"bias0{d}") for d in "fb"}
            for d in "fb":
                nc.sync.dma_start(wih0[d][:], ins[f"wih0{d}"])
                nc.sync.dma_start(whh0[d][:], ins[f"whh0{d}"])
                nc.sync.dma_start(bias0[d][:], ins[f"bias0{d}"])
            zin0 = {d: zpool.tile([128, 8, T, BS], bf16, tag=f"zin{d}", name=f"zin0{d}") for d in "fb"}
            for d in "fb":
                _zin_gemm(nc, mybir, ppool, wih0[d], x0, zin0[d], K0, bias0[d])

        # ---- layer-0 recurrence -> x1 (t-major; chunks f:0-1 b:2-3) ----
        x1 = persist.tile([128, KC1, TB], bf16, tag="x1", name="x1")
        xv1 = x1[:].rearrange("p k (t b) -> p k t b", b=BS)
        with tc.tile_pool(name="ps_l0", bufs=3, space="PSUM") as lstm_ps, \
                tc.tile_pool(name="ps_th0", bufs=1, space="PSUM") as thpool:
            pools["lstm_ps"] = lstm_ps
            pools["th"] = thpool
            _lstm_phase(nc, mybir, pools, whh0, zin0, xv1, ident,
                        [("f", 0), ("b", 2)])

        # ---- layer-1 input GEMMs + recurrence -> x2 (b-major) ----
        wih1 = {d: wpool.tile([128, KC1, G4], bf16, tag=f"wih1{d}", name=f"wih1{d}") for d in "fb"}
        whh1 = {d: wpool.tile([128, 2, G4], bf16, tag=f"whh1{d}", name=f"whh1{d}") for d in "fb"}
        bias1 = {d: wpool.tile([128, 8], f32, tag=f"bias1{d}", name=f"bias1{d}") for d in "fb"}
        for d in "fb":
            nc.sync.dma_start(wih1[d][:], ins[f"wih1{d}"])
            nc.sync.dma_start(whh1[d][:], ins[f"whh1{d}"])
            nc.sync.dma_start(bias1[d][:], ins[f"bias1{d}"])
        zin1 = {d: zpool.tile([128, 8, T, BS], bf16, tag=f"zin{d}", name=f"zin1{d}") for d in "fb"}
        with tc.tile_pool(name="ps_g1", bufs=8, space="PSUM") as ppool:
            for d in "fb":
                _zin_gemm(nc, mybir, ppool, wih1[d], x1, zin1[d], (128,) * KC1, bias1[d])
        x2 = persist.tile([128, 4, TB], bf16, tag="x2", name="x2")
        xv2 = x2[:].rearrange("p k (b t) -> p k t b", t=T)
        with tc.tile_pool(name="ps_l1", bufs=3, space="PSUM") as lstm_ps, \
                tc.tile_pool(name="ps_th1", bufs=1, space="PSUM") as thpool:
            pools["lstm_ps"] = lstm_ps
            pools["th"] = thpool
            _lstm_phase(nc, mybir, pools, whh1, zin1, xv2, ident,
                        [("f", 0), ("b", 2)])

        psum_big = ctx.enter_context(tc.tile_pool(name="psum_tail", bufs=6, space="PSUM"))

        # ---- x2row[(b t), u] via PE transposes ----
        x2row = persist.tile([128, 8, 512], bf16, tag="x2row", name="x2row")
        for c in range(4):
            for j in range(8):
                pst = psum_big.tile([128, 128], bf16, tag="big", name="tr_ps")
                nc.tensor.transpose(pst[:], x2[:, c, j * 128:(j + 1) * 128], ident[:])
                if j % 2 == 0:
                    nc.vector.tensor_copy(out=x2row[:, j, c * 128:(c + 1) * 128],
                                          in_=pst[:])
                else:
                    nc.scalar.copy(x2row[:, j, c * 128:(c + 1) * 128], pst[:])

        # ---- attention: hbar = tanh(ws1 @ x2T) [DA, TB] ----
        hbar = persist.tile([128, TB], bf16, tag="hbar", name="hbar")
        hps = [psum_big.tile([128, 512], f32, tag=f"hb{n}", bufs=1, name="hb_ps")
               for n in range(2)]
        for k in range(4):
            for n in range(2):
                nc.tensor.matmul(hps[n][:], ws1T[:, k, :],
                                 x2[:, k, n * 512:(n + 1) * 512],
                                 start=(k == 0), stop=(k == 3))
        for n in range(2):
            nc.scalar.activation(hbar[:, n * 512:(n + 1) * 512], hps[n][:], AF.Tanh)

        # ---- att[b,r,t] then block-diagonal att2 [(b t), (b r)] ----
        att_ps = psum_big.tile([128, 8, R], f32, tag="big", name="att_ps")
        for bp in range(8):
            nc.tensor.matmul(att_ps[:, bp, :], hbar[:, bp * 128:(bp + 1) * 128],
                             ws2T[:], start=True, stop=True)
        att2 = persist.tile([128, 8, 128], bf16, tag="att2", name="att2")
        nc.vector.memset(att2[:], 0.0)
        for bp in range(8):
            nc.vector.tensor_copy(out=att2[0:64, bp, bp * 16:bp * 16 + 8],
                                  in_=att_ps[0:64, bp, :])
            nc.vector.tensor_copy(out=att2[64:128, bp, bp * 16 + 8:bp * 16 + 16],
                                  in_=att_ps[64:128, bp, :])

        # ---- sentT [u, (b r)] = x2row.T @ att2 ----
        sentT = persist.tile([128, 4, 128], bf16, tag="sentT", name="sentT")
        for c in range(4):
            ps = psum_big.tile([128, 128], f32, tag="big", name="sent_ps")
            for po in range(8):
                nc.tensor.matmul(ps[:], x2row[:, po, c * 128:(c + 1) * 128],
                                 att2[:, po, :], start=(po == 0), stop=(po == 7))
            nc.vector.tensor_copy(out=sentT[:, c, :], in_=ps[:])

        # ---- votes [(r b), (c a)] (bf16 for the routing matmuls) ----
        votes = persist.tile([128, SC * AT], bf16, tag="votes", name="votes")
        vstage = zpool.tile([BS, R, SC * AT], bf16, tag="zinf", name="vstage")
        sentv = sentT[:].rearrange("p k (b r) -> p k r b", r=R)
        dma_eng = [nc.sync, nc.scalar]
        for r in range(R):
            ps = psum_big.tile([BS, 512], f32, tag="big", name="vote_ps")
            for k in range(4):
                nc.tensor.matmul(ps[:], sentv[:, k, r, :], cw[:, r, k, :],
                                 start=(k == 0), stop=(k == 3))
            nc.vector.tensor_copy(out=vstage[:, r, :], in_=ps[:])
        for r in range(R):
            dma_eng[r % 2].dma_start(votes[r * BS:(r + 1) * BS, :], vstage[:, r, :])

        # ---- dynamic routing ----
        ones_pre = persist.tile([128, BS], bf16)
        ones_rep = persist.tile([BS, 128], bf16)
        nc.sync.dma_start(ones_pre[:], ins["ones_pre"])
        nc.sync.dma_start(ones_rep[:], ins["ones_rep"])
        votes_v = votes[:].rearrange("p (c a) -> p c a", a=AT)

        i32 = dt.int32
        MAGIC = 0x5F3759DF

        def rsqrt(pool, x, n):
            """Quake rsqrt on DVE (avoids Sqrt table-set thrash on the
            scalar engine). No Newton step: ~3.4% error only perturbs the
            routing attention weights; the output norm path is exact."""
            t = pool.tile([BS, n], i32, tag="rs_t", name="rs_t")
            nc.vector.tensor_scalar(t[:], x.bitcast(i32), 1, None,
                                    Alu.logical_shift_right)
            j = pool.tile([BS, n], i32, tag="rs_j", name="rs_j")
            nc.vector.tensor_scalar(j[:], t[:], -1, MAGIC, Alu.mult, Alu.add)
            return j[:].bitcast(f32)

        rpool = ctx.enter_context(tc.tile_pool(name="routing", bufs=2))
        logits = None
        n2 = dinv = None
        for it in range(NUM_ROUTING):
            if it == 0:
                route = rpool.tile([128, SC], f32, tag="route", name="route")
                nc.vector.memset(route[:], 1.0 / SC)
            else:
                # logits are O(1): skip the max-subtraction, exp is safe
                e = rpool.tile([128, SC], f32, tag="e", name="e")
                ssum = rpool.tile([128, 1], f32, tag="ssum", name="ssum")
                nc.scalar.activation(e[:], logits[:], AF.Exp, accum_out=ssum[:])
                sinv = rpool.tile([128, 1], f32, tag="sinv", name="sinv")
                nc.vector.reciprocal(sinv[:], ssum[:])
                route = rpool.tile([128, SC], f32, tag="route", name="route")
                nc.vector.tensor_scalar_mul(route[:], e[:], sinv[:])
            tmp = rpool.tile([128, SC, AT], bf16, tag="tmp", name="tmp")
            nc.vector.tensor_tensor(
                tmp[:], votes_v,
                route[:, :, None].to_broadcast((128, SC, AT)), Alu.mult)
            pre = psum_big.tile([BS, SC * AT], f32, tag="big", name="pre_ps")
            nc.tensor.matmul(pre[:], ones_pre[:],
                             tmp[:].rearrange("p c a -> p (c a)"),
                             start=True, stop=True)
            sq = rpool.tile([BS, SC, AT], f32, tag="sq", name="sq")
            nc.scalar.activation(sq[:], pre[:].rearrange("p (c a) -> p c a", a=AT),
                                 AF.Square)
            n2 = rpool.tile([BS, SC], f32, tag="n2", name="n2")
            nc.vector.tensor_reduce(n2[:], sq[:], mybir.AxisListType.X, Alu.add)
            den = rpool.tile([BS, SC], f32, tag="den", name="den")
            nc.vector.tensor_scalar_add(den[:], n2[:], 0.5)
            dinv = rpool.tile([BS, SC], f32, tag="dinv", name="dinv")
            nc.vector.reciprocal(dinv[:], den[:])
            if it < NUM_ROUTING - 1:
                ry = rsqrt(rpool, n2[:], SC)
                s1 = rpool.tile([BS, SC], f32, tag="s1", name="s1")
                nc.vector.tensor_tensor(s1[:], n2[:], dinv[:], Alu.mult)
                nc.vector.tensor_tensor(s1[:], s1[:], ry, Alu.mult)
                act = rpool.tile([BS, SC, AT], bf16, tag="act", name="act")
                nc.vector.tensor_tensor(
                    act[:], pre[:].rearrange("p (c a) -> p c a", a=AT),
                    s1[:, :, None].to_broadcast((BS, SC, AT)), Alu.mult)
                rep = psum_big.tile([128, SC * AT], f32, tag="big", name="rep_ps")
                nc.tensor.matmul(rep[:], ones_rep[:],
                                 act[:].rearrange("p c a -> p (c a)"),
                                 start=True, stop=True)
                u = rpool.tile([128, SC, AT], f32, tag="u", name="u")
                nc.vector.tensor_tensor(
                    u[:], votes_v, rep[:].rearrange("p (c a) -> p c a", a=AT),
                    Alu.mult)
                dl = rpool.tile([128, SC], f32, tag="dl", name="dl")
                nc.vector.tensor_reduce(dl[:], u[:], mybir.AxisListType.X, Alu.add)
                if it == 0:
                    logits = dl
                else:
                    new_logits = rpool.tile([128, SC], f32, tag="logits", name="logits")
                    nc.vector.tensor_add(new_logits[:], logits[:], dl[:])
                    logits = new_logits

        outsb = persist.tile([BS, SC], f32, tag="outsb", name="outsb")
        nc.vector.tensor_tensor(outsb[:], n2[:], dinv[:], Alu.mult)
        nc.sync.dma_start(out, outsb[:])


_CACHED = {}


def _build():
    if "nc" in _CACHED:
        return _CACHED["nc"], _CACHED["ins"]
    import concourse.bacc as bacc
    import concourse.tile as tile
    import concourse.mybir as mybir
    from concourse._compat import axon_active  # noqa: F401

    nc = bacc.Bacc("TRN2", target_bir_lowering=False, debug=False)
    ins, out = _declare_inputs(nc, mybir)
    with tile.TileContext(nc) as tc:
        _body(nc, tc, mybir, ins, out)
    nc.compile()
    _CACHED["nc"] = nc
    _CACHED["ins"] = ins
    return nc, ins


def kernel(**inputs):
    from concourse.bass_utils import run_bass_kernel_spmd

    shared, idx_maps = _host_prep(inputs)
    nc, _ = _build()
    in_maps = []
    for c in range(NCORES):
        m = dict(shared)
        m["idx"] = idx_maps[c]
        in_maps.append(m)
    res = run_bass_kernel_spmd(nc, in_maps, core_ids=list(range(NCORES)))
    out = np.concatenate([res.results[c]["out"] for c in range(NCORES)], axis=0)
    return out.astype(np.float32)


# revision 60
# speedup vs baseline: 1.1911x; 1.0010x over previous
"""CapsuleNetwork (BiLSTM encoder + self-attention pooling + dynamic routing)
as a Trainium2 Bass/Tile kernel, SPMD data-parallel over 8 NeuronCores.

Sharding: batch B=128 split 16/core; weights replicated; no collectives.

Layout: feature dim on SBUF partitions, token index on the free dim.
x0/x1/zin use t-major columns (col = t*BS + b) so each recurrence step is a
contiguous 16-col slice; x2 stays b-major for the attention path.

LSTM step: a PE identity-matmul seeds the PSUM bank with zin (start=True),
the 16 W_hh matmuls accumulate on top (start=False), and one sigmoid over
all 8 gate blocks (g-gate rows pre-scaled x2 host-side; tanh(x)=2sig(2x)-1)
feeds a short elementwise chain.  The two directions are software-pipelined
half a step apart so each direction's matmul block overlaps the other's
elementwise chain.
"""

import sys

sys.path.insert(0, "/opt/trn_rl_repo")

import numpy as np
import ml_dtypes

BF16 = ml_dtypes.bfloat16

# problem dims
B, T, V, E, H, DA, R, SC, AT = 128, 64, 32000, 300, 256, 128, 8, 32, 16
NUM_ROUTING = 3
NCORES = 8
BS = B // NCORES          # 16 examples per core
TB = BS * T               # 1024 columns, t-major: col = t*BS + b
G4 = 4 * H                # 1024 gate rows
K0 = (128, 128, 44)       # layer-0 input chunk sizes (E=300)
KC1 = 4                   # layer-1 input chunks (2H=512)

# torch gate order i,f,g,o -> ours [i,f,o,g] (sigmoid block contiguous)
_PERM = np.concatenate([
    np.arange(0, 256), np.arange(256, 512), np.arange(768, 1024), np.arange(512, 768)
])


def _prep_wih0(w_ih):
    """[4H, 300] -> [128, 3, 1024] bf16 (transposed, gate-permuted, g x2)."""
    w = w_ih[_PERM].T.copy()              # [300, 1024]
    w[:, 768:] *= 2.0                     # sigmoid-trick scaling for g gates
    out = np.zeros((3 * 128, G4), np.float32)
    out[:E] = w
    return np.ascontiguousarray(
        out.reshape(3, 128, G4).transpose(1, 0, 2)).astype(BF16)


def _prep_wih1(w_ih):
    """[4H, 512] -> [128, 4, 1024] bf16."""
    w = w_ih[_PERM].T.copy()              # [512, 1024]
    w[:, 768:] *= 2.0
    return np.ascontiguousarray(
        w.reshape(KC1, 128, G4).transpose(1, 0, 2)).astype(BF16)


def _prep_whh(w_hh):
    """[4H, 256] -> [128, 2, 1024] bf16 (transposed, gate-permuted, g x2)."""
    w = w_hh[_PERM].T.copy()  # [256, 1024]
    w[:, 768:] *= 2.0
    return np.ascontiguousarray(
        w.reshape(2, 128, G4).transpose(1, 0, 2)).astype(BF16)


def _prep_bias(b):
    """[4H] -> [128, 8] f32 per-(m-block) per-partition bias, g x2."""
    bp = b[_PERM].astype(np.float32).copy()
    bp[768:] *= 2.0
    return np.ascontiguousarray(bp.reshape(8, 128).T)


def _host_prep(inputs):
    """Build the shared (replicated) arrays + per-core index arrays."""
    shared = {}

    emb = np.asarray(inputs["embedding"], np.float32)
    shared["emb"] = np.ascontiguousarray(emb).astype(BF16)

    for d, sfx in (("f", "f0"), ("b", "b0")):
        shared[f"wih0{d}"] = _prep_wih0(np.asarray(inputs[f"w_ih_{sfx}"], np.float32))
        shared[f"whh0{d}"] = _prep_whh(np.asarray(inputs[f"w_hh_{sfx}"], np.float32))
        shared[f"bias0{d}"] = _prep_bias(np.asarray(inputs[f"b_{sfx}"], np.float32))
    for d, sfx in (("f", "f1"), ("b", "b1")):
        shared[f"wih1{d}"] = _prep_wih1(np.asarray(inputs[f"w_ih_{sfx}"], np.float32))
        shared[f"whh1{d}"] = _prep_whh(np.asarray(inputs[f"w_hh_{sfx}"], np.float32))
        shared[f"bias1{d}"] = _prep_bias(np.asarray(inputs[f"b_{sfx}"], np.float32))

    ws1 = np.asarray(inputs["ws1"], np.float32)  # [128, 512]
    shared["ws1T"] = np.ascontiguousarray(
        ws1.T.reshape(4, 128, DA).transpose(1, 0, 2)).astype(BF16)
    shared["ws2T"] = np.ascontiguousarray(
        np.asarray(inputs["ws2"], np.float32).T).astype(BF16)  # [128, 8]

    cw = np.asarray(inputs["caps_w"], np.float32)  # [8, 512, 512]
    # -> [128, r=8, k=4, 512]
    shared["cw"] = np.ascontiguousarray(
        cw.reshape(R, 4, 128, SC * AT).transpose(2, 0, 1, 3)).astype(BF16)

    shared["ident"] = np.eye(128, dtype=np.float32).astype(BF16)

    ones_pre = np.zeros((128, BS), np.float32)   # [(r,b), b] block ones
    for r in range(R):
        for b in range(BS):
            ones_pre[r * BS + b, b] = 1.0
    shared["ones_pre"] = ones_pre.astype(BF16)
    shared["ones_rep"] = np.ascontiguousarray(ones_pre.T).astype(BF16)  # [b, (r,b)]

    tokens = np.asarray(inputs["tokens"]).astype(np.int64)  # [128, 64]
    idx_maps = []
    for c in range(NCORES):
        # t-major: token i = t*BS + b  ->  tokens[c*BS + b, t]
        flat = tokens[c * BS:(c + 1) * BS].T.reshape(-1)
        idx_maps.append(np.ascontiguousarray(
            flat.astype(np.int32).reshape(8, 128).T))
    return shared, idx_maps


# ---------------------------------------------------------------------------
# device program
# ---------------------------------------------------------------------------


def _declare_inputs(nc, mybir):
    dt = mybir.dt
    specs = {
        "emb": ((V, E), dt.bfloat16),
        "idx": ((128, 8), dt.int32),
        "wih0f": ((128, 3, G4), dt.bfloat16),
        "wih0b": ((128, 3, G4), dt.bfloat16),
        "whh0f": ((128, 2, G4), dt.bfloat16),
        "whh0b": ((128, 2, G4), dt.bfloat16),
        "bias0f": ((128, 8), dt.float32),
        "bias0b": ((128, 8), dt.float32),
        "wih1f": ((128, KC1, G4), dt.bfloat16),
        "wih1b": ((128, KC1, G4), dt.bfloat16),
        "whh1f": ((128, 2, G4), dt.bfloat16),
        "whh1b": ((128, 2, G4), dt.bfloat16),
        "bias1f": ((128, 8), dt.float32),
        "bias1b": ((128, 8), dt.float32),
        "ws1T": ((128, 4, DA), dt.bfloat16),
        "ws2T": ((128, R), dt.bfloat16),
        "cw": ((128, R, 4, SC * AT), dt.bfloat16),
        "ident": ((128, 128), dt.bfloat16),
        "ones_pre": ((128, BS), dt.bfloat16),
        "ones_rep": ((BS, 128), dt.bfloat16),
    }
    aps = {}
    for name, (shape, dtype) in specs.items():
        aps[name] = nc.dram_tensor(name, list(shape), dtype, kind="ExternalInput").ap()
    out = nc.dram_tensor("out", [BS, SC], mybir.dt.float32, kind="ExternalOutput").ap()
    return aps, out


def _zin_gemm(nc, mybir, ppool, wih_sb, x_sb, zin_sb, chunks, bias_sb):
    """zin[m, t, b] (bf16) = sum_k wihT[k][:,m-blk].T @ xT[k][:, cols] + bias.

    x_sb cols are t-major; zin is m-major so each PSUM half copies contiguous
    into zin[:, m, n*32:(n+1)*32, :].
    """
    f32 = mybir.dt.float32
    Alu = mybir.AluOpType
    AF = mybir.ActivationFunctionType
    nk = len(chunks)
    for m in range(8):
        # n-inner: consecutive matmul pairs share the same stationary
        # weights, halving the LDWEIGHTS pressure on the PE
        pss = [ppool.tile([128, 512], f32, tag=f"big{n}", bufs=3, name="zin_ps")
               for n in range(2)]
        for k, kk in enumerate(chunks):
            for n in range(2):
                nc.tensor.matmul(
                    pss[n][:],
                    wih_sb[0:kk, k, m * 128:(m + 1) * 128],
                    x_sb[0:kk, k, n * 512:(n + 1) * 512],
                    start=(k == 0), stop=(k == nk - 1))
        # bias-added evacuation, alternating engines so neither paces PE
        for n in range(2):
            if (m + n) % 2 == 0:
                nc.vector.tensor_scalar(
                    zin_sb[:, m, n * 32:(n + 1) * 32, :],
                    pss[n][:].rearrange("p (t b) -> p t b", b=BS),
                    bias_sb[:, m:m + 1], None, Alu.add)
            else:
                nc.scalar.activation(
                    zin_sb[:, m, n * 32:(n + 1) * 32, :],
                    pss[n][:].rearrange("p (t b) -> p t b", b=BS),
                    AF.Identity, bias=bias_sb[:, m:m + 1])


def _lstm_phase(nc, mybir, pools, whh_sb, zin_sb, xout_v, ident, dirs):
    """One BiLSTM layer, software-pipelined across the two directions.

    xout_v: [p, k, t, b] view of the output tile. dirs: [(dirname, koff)].
    """
    f32 = mybir.dt.float32
    AF = mybir.ActivationFunctionType
    Alu = mybir.AluOpType
    pspool, spool, state = pools["lstm_ps"], pools["step"], pools["state"]

    cst = {}
    for d, _ in dirs:
        c = state.tile([128, 32], f32, tag=f"c_{id(zin_sb[d])}_{d}",
                       name=f"c_{d}")
        nc.vector.memset(c[:], 0.0)
        cst[d] = c

    ps_t = {}
    ps_q = {d: [] for d, _ in dirs}

    def emit_icopy(d, step):
        # PSUM seed for `step`, emitted one step ahead so it lands in the PE
        # FIFO inside this direction's chain-wait window instead of right
        # before the W-block that depends on h.
        tt = step if d == "f" else T - 1 - step
        ps = pspool.tile([128, 512], f32, tag=f"ps_{d}", name=f"ps_{d}")
        nc.tensor.matmul(ps[:, 0:128], ident[:], zin_sb[d][:, :, tt, :],
                         start=True, stop=(step == 0))
        ps_q[d].append(ps)

    def emit_w_block(d, koff, step):
        tt = step if d == "f" else T - 1 - step
        tprev = tt - 1 if d == "f" else tt + 1
        ps = ps_q[d].pop(0)
        ps_t[d] = ps
        if step > 0:
            for m in range(8):
                for k in range(2):
                    nc.tensor.matmul(
                        ps[:, m * 16:(m + 1) * 16],
                        whh_sb[d][:, k, m * 128:(m + 1) * 128],
                        xout_v[:, koff + k, tprev, :],
                        start=False, stop=(m == 7 and k == 1))

    def emit_ew(d, koff, step):
        tt = step if d == "f" else T - 1 - step
        ps, c = ps_t[d], cst[d]
        s = spool.tile([128, 8, BS], f32, tag=f"s_{d}", name=f"s_{d}")
        nc.scalar.activation(s[:], ps[:, 0:128].rearrange("p (m b) -> p m b", b=BS),
                             AF.Sigmoid)
        t1 = spool.tile([128, 32], f32, tag=f"t1_{d}", name=f"t1_{d}")
        nc.gpsimd.tensor_tensor(
            t1[:], s[:, 2:4, :].rearrange("p m b -> p (m b)"), c[:], Alu.mult)
        # w = (sig_g - 0.5) * sig_i; c = 2w + sig_f*c  ==  sig_i*tanh(g) + sig_f*c
        w = spool.tile([128, 32], f32, tag=f"w_{d}", name=f"w_{d}")
        nc.vector.scalar_tensor_tensor(
            w[:], s[:, 6:8, :].rearrange("p m b -> p (m b)"), 0.5,
            s[:, 0:2, :].rearrange("p m b -> p (m b)"), Alu.subtract, Alu.mult)
        nc.vector.scalar_tensor_tensor(c[:], w[:], 2.0, t1[:], Alu.mult, Alu.add)
        # th lives in PSUM (own bank): ScalarE writes PSUM faster than SBUF,
        # and tanh -> h is on the recurrence critical chain
        th = pools["th"].tile([128, 512], f32, tag=f"th_{d}", name=f"th_{d}")
        nc.scalar.activation(th[:, 0:32], c[:], AF.Tanh)
        nc.vector.tensor_tensor(
            xout_v[:, koff:koff + 2, tt, :], s[:, 4:6, :],
            th[:, 0:32].rearrange("p (m b) -> p m b", b=BS), Alu.mult)

    (df, kf), (db, kb) = dirs
    emit_icopy(df, 0)
    emit_icopy(db, 0)
    for step in range(T):
        emit_w_block(df, kf, step)
        if step > 0:
            emit_ew(db, kb, step - 1)
        emit_w_block(db, kb, step)
        # both seeds back-to-back on the PE so the second shares the
        # identity LDWEIGHTS with the first
        if step + 1 < T:
            emit_icopy(df, step + 1)
            emit_icopy(db, step + 1)
        emit_ew(df, kf, step)
    emit_ew(db, kb, T - 1)


def _body(nc, tc, mybir, ins, out):
    import contextlib
    dt = mybir.dt
    AF = mybir.ActivationFunctionType
    Alu = mybir.AluOpType
    bf16, f32 = dt.bfloat16, dt.float32

    ctx = contextlib.ExitStack()
    with ctx:
        persist = ctx.enter_context(tc.tile_pool(name="persist", bufs=1))
        wpool = ctx.enter_context(tc.tile_pool(name="weights", bufs=1))
        zpool = ctx.enter_context(tc.tile_pool(name="zin", bufs=1))
        state = ctx.enter_context(tc.tile_pool(name="state", bufs=1))
        step = ctx.enter_context(tc.tile_pool(name="step", bufs=3))
        pools = {"state": state, "step": step}

        # ---- load indices + gather embedding rows, transpose on PE ----
        import concourse.bass as bass
        idx_sb = persist.tile([128, 8], dt.int32)
        nc.sync.dma_start(idx_sb[:], ins["idx"])
        ident = persist.tile([128, 128], bf16)
        nc.sync.dma_start(ident[:], ins["ident"])
        # prefetch the big capsule weights early so they overlap the LSTMs
        cw = persist.tile([128, R, 4, SC * AT], bf16, tag="cw", name="cw")
        nc.scalar.dma_start(cw[:], ins["cw"])
        ws1T = persist.tile([128, 4, DA], bf16)
        ws2T = persist.tile([128, R], bf16)
        nc.sync.dma_start(ws1T[:], ins["ws1T"])
        nc.sync.dma_start(ws2T[:], ins["ws2T"])

        # tiny dummy sigmoid first so the initial ACT table load picks the
        # sigmoid set (avoids a ~2.7us set switch at the LSTM0 boundary)
        warm = persist.tile([1, 1], f32, tag="warm", name="warm")
        nc.vector.memset(warm[:], 0.0)
        nc.scalar.activation(warm[:], warm[:], AF.Sigmoid)

        x0rows = persist.tile([128, 8, E], bf16, tag="x0rows", name="x0rows")
        for j in range(8):
            nc.gpsimd.indirect_dma_start(
                out=x0rows[:, j, :], out_offset=None, in_=ins["emb"],
                in_offset=bass.IndirectOffsetOnAxis(ap=idx_sb[:, j:j + 1], axis=0))

        with tc.tile_pool(name="ps_g0", bufs=6, space="PSUM") as ppool:
            x0 = persist.tile([128, 3, TB], bf16, tag="x0", name="x0")
            # c-outer: bunches the transposes ahead of the GEMM so the GEMM
            # runs as one dense matmul burst (PE transposes don't count as
            # busy for the HAM clock gate; interleaving them keeps PE cold)
            for c in range(3):
                kk = K0[c]
                for j in range(8):
                    pst = ppool.tile([128, 128], bf16, tag="tr", bufs=2, name="g_tr")
                    nc.tensor.transpose(pst[0:kk, :],
                                        x0rows[:, j, c * 128:c * 128 + kk], ident[:])
                    nc.vector.tensor_copy(out=x0[0:kk, c, j * 128:(j + 1) * 128],
                                          in_=pst[0:kk, :])

            # ---- layer-0 weights + input GEMMs ----
            wih0 = {d: wpool.tile([128, 3, G4], bf16, tag=f"wih0{d}", name=f"wih0{d}") for d in "fb"}
            whh0 = {d: wpool.tile([128, 2, G4], bf16, tag=f"whh0{d}", name=f"whh0{d}") for d in "fb"}
            bias0 = {d: wpool.tile([128, 8], f32, tag=f"bias0{d}", name=f"bias0{d}") for d in "fb"}
            for d in "fb":
                nc.sync.dma_start(wih0[d][:], ins[f"wih0{d}"])
                nc.sync.dma_start(whh0[d][:], ins[f"whh0{d}"])
                nc.sync.dma_start(bias0[d][:], ins[f"bias0{d}"])
            zin0 = {d: zpool.tile([128, 8, T, BS], bf16, tag=f"zin{d}", name=f"zin0{d}") for d in "fb"}
            for d in "fb":
                _zin_gemm(nc, mybir, ppool, wih0[d], x0, zin0[d], K0, bias0[d])

        # ---- layer-0 recurrence -> x1 (t-major; chunks f:0-1 b:2-3) ----
        x1 = persist.tile([128, KC1, TB], bf16, tag="x1", name="x1")
        xv1 = x1[:].rearrange("p k (t b) -> p k t b", b=BS)
        with tc.tile_pool(name="ps_l0", bufs=3, space="PSUM") as lstm_ps, \
                tc.tile_pool(name="ps_th0", bufs=1, space="PSUM") as thpool:
            pools["lstm_ps"] = lstm_ps
            pools["th"] = thpool
            _lstm_phase(nc, mybir, pools, whh0, zin0, xv1, ident,
                        [("f", 0), ("b", 2)])

        # ---- layer-1 input GEMMs + recurrence -> x2 (b-major) ----
        wih1 = {d: wpool.tile([128, KC1, G4], bf16, tag=f"wih1{d}", name=f"wih1{d}") for d in "fb"}
        whh1 = {d: wpool.tile([128, 2, G4], bf16, tag=f"whh1{d}", name=f"whh1{d}") for d in "fb"}
        bias1 = {d: wpool.tile([128, 8], f32, tag=f"bias1{d}", name=f"bias1{d}") for d in "fb"}
        for d in "fb":
            nc.sync.dma_start(wih1[d][:], ins[f"wih1{d}"])
            nc.sync.dma_start(whh1[d][:], ins[f"whh1{d}"])
            nc.sync.dma_start(bias1[d][:], ins[f"bias1{d}"])
        zin1 = {d: zpool.tile([128, 8, T, BS], bf16, tag=f"zin{d}", name=f"zin1{d}") for d in "fb"}
        with tc.tile_pool(name="ps_g1", bufs=8, space="PSUM") as ppool:
            for d in "fb":
                _zin_gemm(nc, mybir, ppool, wih1[d], x1, zin1[d], (128,) * KC1, bias1[d])
        x2 = persist.tile([128, 4, TB], bf16, tag="x2", name="x2")
        xv2 = x2[:].rearrange("p k (b t) -> p k t b", t=T)
        with tc.tile_pool(name="ps_l1", bufs=3, space="PSUM") as lstm_ps, \
                tc.tile_pool(name="ps_th1", bufs=1, space="PSUM") as thpool:
            pools["lstm_ps"] = lstm_ps
            pools["th"] = thpool
            _lstm_phase(nc, mybir, pools, whh1, zin1, xv2, ident,
                        [("f", 0), ("b", 2)])

        psum_big = ctx.enter_context(tc.tile_pool(name="psum_tail", bufs=6, space="PSUM"))

        # ---- x2row[(b t), u] via PE transposes ----
        x2row = persist.tile([128, 8, 512], bf16, tag="x2row", name="x2row")
        for c in range(4):
            for j in range(8):
                pst = psum_big.tile([128, 128], bf16, tag="big", name="tr_ps")
                nc.tensor.transpose(pst[:], x2[:, c, j * 128:(j + 1) * 128], ident[:])
                if j % 2 == 0:
                    nc.vector.tensor_copy(out=x2row[:, j, c * 128:(c + 1) * 128],
                                          in_=pst[:])
                else:
                    nc.scalar.copy(x2row[:, j, c * 128:(c + 1) * 128], pst[:])

        # ---- attention: hbar = tanh(ws1 @ x2T) [DA, TB] ----
        hbar = persist.tile([128, TB], bf16, tag="hbar", name="hbar")
        hps = [psum_big.tile([128, 512], f32, tag=f"hb{n}", bufs=1, name="hb_ps")
               for n in range(2)]
        for k in range(4):
            for n in range(2):
                nc.tensor.matmul(hps[n][:], ws1T[:, k, :],
                                 x2[:, k, n * 512:(n + 1) * 512],
                                 start=(k == 0), stop=(k == 3))
        for n in range(2):
            nc.scalar.activation(hbar[:, n * 512:(n + 1) * 512], hps[n][:], AF.Tanh)

        # ---- att[b,r,t] then block-diagonal att2 [(b t), (b r)] ----
        att_ps = psum_big.tile([128, 8, R], f32, tag="big", name="att_ps")
        for bp in range(8):
            nc.tensor.matmul(att_ps[:, bp, :], hbar[:, bp * 128:(bp + 1) * 128],
                             ws2T[:], start=True, stop=True)
        att2 = persist.tile([128, 8, 128], bf16, tag="att2", name="att2")
        nc.vector.memset(att2[:], 0.0)
        for bp in range(8):
            nc.vector.tensor_copy(out=att2[0:64, bp, bp * 16:bp * 16 + 8],
                                  in_=att_ps[0:64, bp, :])
            nc.vector.tensor_copy(out=att2[64:128, bp, bp * 16 + 8:bp * 16 + 16],
                                  in_=att_ps[64:128, bp, :])

        # ---- sentT [u, (b r)] = x2row.T @ att2 ----
        sentT = persist.tile([128, 4, 128], bf16, tag="sentT", name="sentT")
        for c in range(4):
            ps = psum_big.tile([128, 128], f32, tag="big", name="sent_ps")
            for po in range(8):
                nc.tensor.matmul(ps[:], x2row[:, po, c * 128:(c + 1) * 128],
                                 att2[:, po, :], start=(po == 0), stop=(po == 7))
            nc.vector.tensor_copy(out=sentT[:, c, :], in_=ps[:])

        # ---- votes [(r b), (c a)] (bf16 for the routing matmuls) ----
        votes = persist.tile([128, SC * AT], bf16, tag="votes", name="votes")
        vstage = zpool.tile([BS, R, SC * AT], bf16, tag="zinf", name="vstage")
        sentv = sentT[:].rearrange("p k (b r) -> p k r b", r=R)
        dma_eng = [nc.sync, nc.scalar]
        for r in range(R):
            ps = psum_big.tile([BS, 512], f32, tag="big", name="vote_ps")
            for k in range(4):
                nc.tensor.matmul(ps[:], sentv[:, k, r, :], cw[:, r, k, :],
                                 start=(k == 0), stop=(k == 3))
            nc.vector.tensor_copy(out=vstage[:, r, :], in_=ps[:])
        for r in range(R):
            dma_eng[r % 2].dma_start(votes[r * BS:(r + 1) * BS, :], vstage[:, r, :])

        # ---- dynamic routing ----
        ones_pre = persist.tile([128, BS], bf16)
        ones_rep = persist.tile([BS, 128], bf16)
        nc.sync.dma_start(ones_pre[:], ins["ones_pre"])
        nc.sync.dma_start(ones_rep[:], ins["ones_rep"])
        votes_v = votes[:].rearrange("p (c a) -> p c a", a=AT)

        i32 = dt.int32
        MAGIC = 0x5F3759DF

        def rsqrt(pool, x, n):
            """Quake rsqrt + one Newton step, all on DVE (avoids Sqrt
            table-set thrash on the scalar engine)."""
            t = pool.tile([BS, n], i32, tag="rs_t", name="rs_t")
            nc.vector.tensor_scalar(t[:], x.bitcast(i32), 1, None,
                                    Alu.logical_shift_right)
            j = pool.tile([BS, n], i32, tag="rs_j", name="rs_j")
            nc.vector.tensor_scalar(j[:], t[:], -1, MAGIC, Alu.mult, Alu.add)
            y = j[:].bitcast(f32)
            a = pool.tile([BS, n], f32, tag="rs_a", name="rs_a")
            nc.vector.tensor_tensor(a[:], y, y, Alu.mult)
            nc.vector.tensor_tensor(a[:], a[:], x, Alu.mult)
            nc.vector.tensor_scalar(a[:], a[:], -0.5, 1.5, Alu.mult, Alu.add)
            o = pool.tile([BS, n], f32, tag="rs_o", name="rs_o")
            nc.vector.tensor_tensor(o[:], y, a[:], Alu.mult)
            return o

        rpool = ctx.enter_context(tc.tile_pool(name="routing", bufs=2))
        logits = None
        n2 = dinv = None
        for it in range(NUM_ROUTING):
            if it == 0:
                route = rpool.tile([128, SC], f32, tag="route", name="route")
                nc.vector.memset(route[:], 1.0 / SC)
            else:
                # logits are O(1): skip the max-subtraction, exp is safe
                e = rpool.tile([128, SC], f32, tag="e", name="e")
                ssum = rpool.tile([128, 1], f32, tag="ssum", name="ssum")
                nc.scalar.activation(e[:], logits[:], AF.Exp, accum_out=ssum[:])
                sinv = rpool.tile([128, 1], f32, tag="sinv", name="sinv")
                nc.vector.reciprocal(sinv[:], ssum[:])
                route = rpool.tile([128, SC], f32, tag="route", name="route")
                nc.vector.tensor_scalar_mul(route[:], e[:], sinv[:])
            tmp = rpool.tile([128, SC, AT], bf16, tag="tmp", name="tmp")
            nc.vector.tensor_tensor(
                tmp[:], votes_v,
                route[:, :, None].to_broadcast((128, SC, AT)), Alu.mult)
            pre = psum_big.tile([BS, SC * AT], f32, tag="big", name="pre_ps")
            nc.tensor.matmul(pre[:], ones_pre[:],
                             tmp[:].rearrange("p c a -> p (c a)"),
                             start=True, stop=True)
            sq = rpool.tile([BS, SC, AT], f32, tag="sq", name="sq")
            nc.scalar.activation(sq[:], pre[:].rearrange("p (c a) -> p c a", a=AT),
                                 AF.Square)
            n2 = rpool.tile([BS, SC], f32, tag="n2", name="n2")
            nc.vector.tensor_reduce(n2[:], sq[:], mybir.AxisListType.X, Alu.add)
            den = rpool.tile([BS, SC], f32, tag="den", name="den")
            nc.vector.tensor_scalar_add(den[:], n2[:], 0.5)
            dinv = rpool.tile([BS, SC], f32, tag="dinv", name="dinv")
            nc.vector.reciprocal(dinv[:], den[:])
            if it < NUM_ROUTING - 1:
                ry = rsqrt(rpool, n2[:], SC)
                s1 = rpool.tile([BS, SC], f32, tag="s1", name="s1")
                nc.vector.tensor_tensor(s1[:], n2[:], dinv[:], Alu.mult)
                nc.vector.tensor_tensor(s1[:], s1[:], ry[:], Alu.mult)
                act = rpool.tile([BS, SC, AT], bf16, tag="act", name="act")
                nc.vector.tensor_tensor(
                    act[:], pre[:].rearrange("p (c a) -> p c a", a=AT),
                    s1[:, :, None].to_broadcast((BS, SC, AT)), Alu.mult)
                rep = psum_big.tile([128, SC * AT], f32, tag="big", name="rep_ps")
                nc.tensor.matmul(rep[:], ones_rep[:],
                                 act[:].rearrange("p c a -> p (c a)"),
                                 start=True, stop=True)
                u = rpool.tile([128, SC, AT], f32, tag="u", name="u")
                nc.vector.tensor_tensor(
                    u[:], votes_v, rep[:].rearrange("p (c a) -> p c a", a=AT),
                    Alu.mult)
                dl = rpool.tile([128, SC], f32, tag="dl", name="dl")
                nc.vector.tensor_reduce(dl[:], u[:], mybir.AxisListType.X, Alu.add)
                if it == 0:
                    logits = dl
                else:
                    new_logits = rpool.tile([128, SC], f32, tag="logits", name="logits")
                    nc.vector.tensor_add(new_logits[:], logits[:], dl[:])
                    logits = new_logits

        outsb = persist.tile([BS, SC], f32, tag="outsb", name="outsb")
        nc.vector.tensor_tensor(outsb[:], n2[:], dinv[:], Alu.mult)
        nc.sync.dma_start(out, outsb[:])


_CACHED = {}


def _build():
    if "nc" in _CACHED:
        return _CACHED["nc"], _CACHED["ins"]
    import concourse.bacc as bacc
    import concourse.tile as tile
    import concourse.mybir as mybir
    from concourse._compat import axon_active  # noqa: F401

    nc = bacc.Bacc("TRN2", target_bir_lowering=False, debug=False)
    ins, out = _declare_inputs(nc, mybir)
    with tile.TileContext(nc) as tc:
        _body(nc, tc, mybir, ins, out)
    nc.compile()
    _CACHED["nc"] = nc
    _CACHED["ins"] = ins
    return nc, ins


def kernel(**inputs):
    from concourse.bass_utils import run_bass_kernel_spmd

    shared, idx_maps = _host_prep(inputs)
    nc, _ = _build()
    in_maps = []
    for c in range(NCORES):
        m = dict(shared)
        m["idx"] = idx_maps[c]
        in_maps.append(m)
    res = run_bass_kernel_spmd(nc, in_maps, core_ids=list(range(NCORES)))
    out = np.concatenate([res.results[c]["out"] for c in range(NCORES)], axis=0)
    return out.astype(np.float32)
